# revision 87
# baseline (speedup 1.0000x reference)
"""Batch multi-head graph attention (GAT) kernel for 8 Trainium2 NeuronCores.

Reference computation (per batch b, head h; n=1024 nodes, f_in=128, f_out=64):
    hp      = h @ w[h]                              # [n, 64]
    t       = tanh(hp)
    src     = t @ a_src[h];  dst = t @ a_dst[h]     # [n]
    score   = leaky_relu(src[i] + dst[j], 0.2)
    attn    = softmax over j of score masked by adj[i, j] > 0
    out     = attn @ hp + bias

Kernel reformulation (exact, modulo fp):
    exp(leaky(x)) = max(exp(x), exp(0.2 x))   (exp monotone, leaky = max(x, .2x))
    with x = src_i + dst_j both branches are rank-1 separable. Dividing by the
    row constant exp(0.2 src_i) (cancels in softmax):
        m[j, i] = adjT[j, i] * max(P_i * H_j, F_j)
        P = exp(0.8 src), H = exp(dst), F = exp(0.2 dst)
    out[i, :] = (sum_j m[j,i] * hp[j,:]) / (sum_j m[j,i]) + bias
    The denominator comes free as a ones-column appended to hp; bias is added
    on the host.

Sharding: data-parallel over batch (16 -> 2 per core); params replicated.
"""

from contextlib import ExitStack

import numpy as np

import concourse.tile as tile
from concourse import bacc, mybir
from concourse._compat import with_exitstack
from concourse.bass_utils import run_bass_kernel_spmd
from concourse.masks import make_identity

F32 = mybir.dt.float32
F16 = mybir.dt.float16
F32R = mybir.dt.float32r
I32 = mybir.dt.int32
AL = mybir.AluOpType
AF = mybir.ActivationFunctionType

N_CORES = 8
BS = 16
B_PER_CORE = BS // N_CORES  # 2
N = 1024
F_IN = 128
F_OUT = 64
N_HEAD = 4
NCH = N // 128  # 8 chunks of 128 nodes


@with_exitstack
def _gat_tile_kernel(ctx: ExitStack, tc: tile.TileContext, out_ap, hT_ap, adjT_ap,
                     w_ap, a_ap):
    nc = tc.nc

    singles = ctx.enter_context(tc.tile_pool(name="singles", bufs=1))
    hT_pool = ctx.enter_context(tc.tile_pool(name="hT", bufs=2))
    adjf_pool = ctx.enter_context(tc.tile_pool(name="adjf", bufs=3))
    tT_pool = ctx.enter_context(tc.tile_pool(name="tT", bufs=4))
    rows_pool = ctx.enter_context(tc.tile_pool(name="rows", bufs=1))
    small_pool = ctx.enter_context(tc.tile_pool(name="small", bufs=4))
    hfcol_pool = ctx.enter_context(tc.tile_pool(name="hfcol", bufs=10))
    pb_pool = ctx.enter_context(tc.tile_pool(name="pb", bufs=6))
    u_pool = ctx.enter_context(tc.tile_pool(name="u", bufs=6))
    m_pool = ctx.enter_context(tc.tile_pool(name="m", bufs=16))
    haug_pool = ctx.enter_context(tc.tile_pool(name="haug", bufs=2))
    oT_pool = ctx.enter_context(tc.tile_pool(name="oTp", bufs=3))
    outp_pool = ctx.enter_context(tc.tile_pool(name="outp", bufs=2))

    dram_pool = ctx.enter_context(tc.tile_pool(name="dram", bufs=2, space="DRAM"))
    ps_hpT = ctx.enter_context(tc.tile_pool(name="ps_hpT", bufs=1, space="PSUM"))
    ps_hp = ctx.enter_context(tc.tile_pool(name="ps_hp", bufs=1, space="PSUM"))
    ps_rows = ctx.enter_context(tc.tile_pool(name="ps_rows", bufs=1, space="PSUM"))
    ps_rc = ctx.enter_context(tc.tile_pool(name="ps_rc", bufs=1, space="PSUM"))
    ps_oT = ctx.enter_context(tc.tile_pool(name="ps_oT", bufs=2, space="PSUM"))
    ps_ot2 = ctx.enter_context(tc.tile_pool(name="ps_ot2", bufs=2, space="PSUM"))

    # ---- constants / params (loaded once) ----
    ident65 = singles.tile([65, 65], F32)
    make_identity(nc, ident65[:])

    w_sb = singles.tile([F_IN, N_HEAD, F_OUT], F32)  # [f_in, h, o]
    nc.sync.dma_start(w_sb[:], w_ap.rearrange("h f o -> f h o"))

    a_sb = singles.tile([F_OUT, N_HEAD, 2], F32)  # [o, h, (src,dst)]
    nc.sync.dma_start(a_sb[:], a_ap.rearrange("h o s -> o h s"))

    # f32r-rounded copies (PE fast-fp32 path requires rounded producers)
    wr = singles.tile([F_IN, N_HEAD, F_OUT], F32R)
    nc.vector.tensor_copy(wr[:], w_sb[:])

    def a_al(h, sl):
        return a_sb[0:F_OUT, h, sl]

    def emit_loads(b):
        # ---- h (host-pretransposed [f_in, n]) and adjacency (host [j, i]) ----
        hT_sb = hT_pool.tile([F_IN, N], F32, tag="hT", name=f"hT_sb{b}")
        nc.gpsimd.dma_start(hT_sb[:], hT_ap[b])
        hTr = hT_pool.tile([F_IN, N], F32R, tag="hTr", name=f"hTr{b}")
        nc.vector.tensor_copy(hTr[:], hT_sb[:])

        adj_f16 = []  # 2 big f16 tiles, 4 j-chunks each; DMA casts i32->f16
        for half in range(2):
            af = adjf_pool.tile([128, 4, N], F16, tag="adjf", name=f"adjf{b}{half}")
            for q in range(2):
                ofs = half * 512 + q * 256
                nc.gpsimd.dma_start(
                    af[:, q * 2:(q + 1) * 2, :],
                    adjT_ap[b, ofs:ofs + 256, :].rearrange("(c p) i -> p c i",
                                                           p=128))
            adj_f16.append(af)
        return dict(hT_sb=hT_sb, hTr=hTr, adj_f16=adj_f16)

    def emit_prep(b, ld):
        hT_sb, hTr, adj_f16 = ld["hT_sb"], ld["hTr"], ld["adj_f16"]

        # ---- h_primeT per head (for tanh/tT); h_prime natural all heads ----
        hp_ctx = tc.high_priority()
        hp_ctx.__enter__()
        tTs = [tT_pool.tile([F_OUT, N], F32, tag="tT", name=f"tT{b}{h}")
               for h in range(N_HEAD)]
        for c in range(2):  # fp32 moving limit 512
            for h in range(N_HEAD):
                p = ps_hpT.tile([F_OUT, 512], F32, tag="p", name=f"p{b}{c}{h}")
                nc.tensor.matmul(p[:], wr[:, h, :],
                                 hTr[:, c * 512:(c + 1) * 512],
                                 start=True, stop=True)
                nc.scalar.activation(tTs[h][:, c * 512:(c + 1) * 512], p[:],
                                     AF.Tanh)

        # ---- src rows (for P broadcast): M=1 col-tiled matmuls at base 32h ----
        # h-major so head 0's full chain (rows -> exp -> broadcast) clears
        # before later heads' work occupies ACT.
        prow = rows_pool.tile([1, N_HEAD * N], F16, tag="prow",
                              name=f"prow{b}")  # 4 heads concat
        p_dram = dram_pool.tile([1, N_HEAD * N], F16, tag="pdram", name=f"pdram{b}")
        pbs = []
        for c in range(2):
            pr_s = ps_rows.tile([128, 512], F32, tag="pr_s", name=f"pr_s{b}{c}")
            for h in range(N_HEAD):
                nc.tensor.matmul(pr_s[32 * h:32 * h + 1, :],
                                 a_al(h, slice(0, 1)),
                                 tTs[h][:, c * 512:(c + 1) * 512],
                                 start=True, stop=True,
                                 tile_position=(0, 32 * h))
            for h in range(N_HEAD):
                nc.scalar.activation(
                    prow[0:1, h * N + c * 512:h * N + (c + 1) * 512],
                    pr_s[32 * h:32 * h + 1, :], AF.Exp, scale=0.8)
                lo = h * N + c * 512
                nc.sync.dma_start(p_dram[0:1, lo:lo + 512],
                                  prow[0:1, lo:lo + 512])
                if c == 0:
                    pb = pb_pool.tile([128, N], F16, tag="pb4", name=f"pb{b}{h}")
                    pbs.append(pb)
                nc.sync.dma_start(
                    pbs[h][:, c * 512:(c + 1) * 512],
                    p_dram[0:1, lo:lo + 512].to_broadcast([128, 512]))

        # ---- dst columns per j-chunk: lhsT = tT chunk, rhs = a vectors ----
        # hfcols[jc][:, 0:4] = H = exp(dst) per head; [:, 4:8] = F = exp(.2 dst)
        # First two chunks exp per-head so pass1(h0) unblocks early.
        hfcols = []
        for jc in range(NCH):
            prc = ps_rc.tile([128, N_HEAD, 2], F32, tag="prc", name=f"prc{b}{jc}")
            for h in range(N_HEAD):
                nc.tensor.matmul(prc[:, h, :], tTs[h][:, jc * 128:(jc + 1) * 128],
                                 a_al(h, slice(0, 2)), start=True, stop=True)
            hc = hfcol_pool.tile([128, 8], F32, tag="hfcol", name=f"hfcol{b}{jc}")
            if jc < 2:
                for h in range(N_HEAD):
                    nc.scalar.activation(hc[:, h:h + 1], prc[:, h, 1:2],
                                         AF.Exp, scale=1.0)
                    nc.scalar.activation(hc[:, 4 + h:5 + h], prc[:, h, 1:2],
                                         AF.Exp, scale=0.2)
            else:
                nc.scalar.activation(hc[:, 0:4], prc[:, :, 1], AF.Exp, scale=1.0)
                nc.scalar.activation(hc[:, 4:8], prc[:, :, 1], AF.Exp, scale=0.2)
            hfcols.append(hc)
        hp_ctx.__exit__(None, None, None)

        ot4 = outp_pool.tile([128, N_HEAD, NCH, F_OUT], F32, tag="ot4",
                             name=f"ot4{b}")
        return dict(b=b, adj_f16=adj_f16, pbs=pbs, hfcols=hfcols,
                    ot4=ot4, hTr=hTr)

    def emit_haug(b, hTr):
        # haug4[j, jc, h, 0:64] = h_prime; [..., 64] = 1.0 (denominator column)
        # Emitted AFTER the first score pass so the DVE copies don't
        # head-of-line block pass1; b1's build is hoisted into b0's ramp.
        haug4 = haug_pool.tile([128, NCH, N_HEAD, F_OUT + 1], F16, tag="haug4",
                               name=f"haug4{b}")
        nc.gpsimd.memset(haug4[:, :, :, F_OUT:F_OUT + 1], 1.0)
        for ic in range(NCH):
            php = ps_hp.tile([128, N_HEAD * F_OUT], F32, tag="php",
                             name=f"php{b}{ic}")
            nc.tensor.matmul(php[:],
                             hTr[:, ic * 128:(ic + 1) * 128],
                             wr[:].rearrange("f h o -> f (h o)"),
                             start=True, stop=True)
            nc.vector.tensor_copy(haug4[:, ic, :, 0:F_OUT],
                                  php[:].rearrange("p (h o) -> p h o", h=N_HEAD))
        return haug4

    def emit_score(st, h):
        # ---- score passes over the [n, n] tensor (DVE) ----
        b, pbs, hfcols = st["b"], st["pbs"], st["hfcols"]
        ms = []
        for jc in range(NCH):
            u = u_pool.tile([128, N], F16, tag="u", name=f"u{b}{h}{jc}")
            nc.vector.tensor_scalar(u[:], pbs[h][:],
                                    hfcols[jc][:, h:h + 1],
                                    hfcols[jc][:, 4 + h:5 + h],
                                    AL.mult, AL.max)
            m = m_pool.tile([128, N], F16, tag="m", name=f"m{b}{h}{jc}")
            af = st["adj_f16"][jc // 4][:, jc % 4, :]
            eng = nc.gpsimd if jc < 2 else nc.vector
            eng.tensor_tensor(m[:], u[:], af, AL.mult)
            ms.append(m)
        return ms

    def emit_agg(st, h, ms):
        # ---- aggregation (transposed): oT[o, i] = sum_j haug[j,o] m[j,i] ----
        b, haug4, ot4 = st["b"], st["haug4"], st["ot4"]
        for half in range(2):
            poT = ps_oT.tile([F_OUT + 1, 512], F32, tag="poT",
                             name=f"poT{b}{h}{half}")
            for jc in range(NCH):
                nc.tensor.matmul(poT[:], haug4[:, jc, h, :],
                                 ms[jc][:, half * 512:(half + 1) * 512],
                                 start=(jc == 0), stop=(jc == NCH - 1))
            oT_sb = oT_pool.tile([F_OUT + 1, 512], F32, tag="oT_sb",
                                 name=f"oT_sb{b}{h}{half}")
            nc.scalar.copy(oT_sb[:], poT[:])
            po2 = ps_ot2.tile([128, 4, F_OUT + 1], F32, tag="po2",
                              name=f"po2{b}{h}{half}")
            for q in range(4):
                nc.tensor.transpose(po2[:, q, :], oT_sb[:, q * 128:(q + 1) * 128],
                                    ident65[:])
            rden = small_pool.tile([128, 4], F32, tag="rden",
                                   name=f"rden{b}{h}{half}")
            nc.vector.reciprocal(rden[:], po2[:, :, F_OUT])
            for q in range(4):
                ic = 4 * half + q
                nc.scalar.mul(ot4[:, h, ic, :], po2[:, q, 0:F_OUT],
                              rden[:, q:q + 1])

    def emit_score_agg_fused(st, h):
        # per-jc: score passes then immediately the two agg matmuls, so the
        # tail unit's aggregation tracks DVE instead of trailing it
        b, pbs, hfcols, haug4 = st["b"], st["pbs"], st["hfcols"], st["haug4"]
        poT0 = ps_oT.tile([F_OUT + 1, 512], F32, tag="poT", name=f"poTf{b}{h}0")
        poT1 = ps_oT.tile([F_OUT + 1, 512], F32, tag="poT", name=f"poTf{b}{h}1")
        for jc in range(NCH):
            u = u_pool.tile([128, N], F16, tag="u", name=f"uf{b}{h}{jc}")
            nc.vector.tensor_scalar(u[:], pbs[h][:],
                                    hfcols[jc][:, h:h + 1],
                                    hfcols[jc][:, 4 + h:5 + h],
                                    AL.mult, AL.max)
            m = m_pool.tile([128, N], F16, tag="m", name=f"mf{b}{h}{jc}")
            af = st["adj_f16"][jc // 4][:, jc % 4, :]
            eng = nc.gpsimd if jc < 2 else nc.vector
            eng.tensor_tensor(m[:], u[:], af, AL.mult)
            nc.tensor.matmul(poT0[:], haug4[:, jc, h, :], m[:, 0:512],
                             start=(jc == 0), stop=(jc == NCH - 1))
            nc.tensor.matmul(poT1[:], haug4[:, jc, h, :], m[:, 512:1024],
                             start=(jc == 0), stop=(jc == NCH - 1))
        for half, poT in ((0, poT0), (1, poT1)):
            oT_sb = oT_pool.tile([F_OUT + 1, 512], F32, tag="oT_sb",
                                 name=f"oT_sbf{b}{h}{half}")
            nc.scalar.copy(oT_sb[:], poT[:])
            po2 = ps_ot2.tile([128, 4, F_OUT + 1], F32, tag="po2",
                              name=f"po2f{b}{h}{half}")
            for q in range(4):
                nc.tensor.transpose(po2[:, q, :], oT_sb[:, q * 128:(q + 1) * 128],
                                    ident65[:])
            rden = small_pool.tile([128, 4], F32, tag="rden",
                                   name=f"rdenf{b}{h}{half}")
            nc.vector.reciprocal(rden[:], po2[:, :, F_OUT])
            for q in range(4):
                ic = 4 * half + q
                nc.scalar.mul(st["ot4"][:, h, ic, :], po2[:, q, 0:F_OUT],
                              rden[:, q:q + 1])
            nc.sync.dma_start(
                out_ap[st["b"], :, h, 4 * half:4 * half + 4],
                st["ot4"][:, h, 4 * half:4 * half + 4])

    def emit_out_h(st, h):
        nc.sync.dma_start(out_ap[st["b"], :, h], st["ot4"][:, h])

    # Software-pipelined emission: score passes (DVE) run ahead of
    # aggregation so no engine head-of-line blocks on a cross-engine dep.
    LAG = 2
    queue = []  # (state, h, ms) awaiting aggregation
    loads = [emit_loads(b) for b in range(B_PER_CORE)]
    haug_cache = {}
    for b in range(B_PER_CORE):
        st = emit_prep(b, loads[b])
        if b == 0:
            # both haug builds go into the ramp: php matmuls depend only on
            # hTr, ready long before the first P broadcast gates pass1, so
            # the DVE copies fill otherwise-idle ramp cycles
            for bb in range(B_PER_CORE):
                haug_cache[bb] = emit_haug(bb, loads[bb]["hTr"])
        st["haug4"] = haug_cache[b]
        last = b == B_PER_CORE - 1
        for h in range(N_HEAD):
            if last and h == N_HEAD - 1:
                # drain the queue first so the tail is only this unit
                while queue:
                    qst, qh, qms = queue.pop(0)
                    emit_agg(qst, qh, qms)
                    emit_out_h(qst, qh)
                emit_score_agg_fused(st, h)
                break
            queue.append((st, h, emit_score(st, h)))
            if len(queue) > LAG:
                qst, qh, qms = queue.pop(0)
                emit_agg(qst, qh, qms)
                emit_out_h(qst, qh)
    while queue:
        qst, qh, qms = queue.pop(0)
        emit_agg(qst, qh, qms)
        emit_out_h(qst, qh)


def _build_nc():
    nc = bacc.Bacc("TRN2", target_bir_lowering=False, debug=False,
                   num_devices=N_CORES)
    hT = nc.dram_tensor("hT", [B_PER_CORE, F_IN, N], F32, kind="ExternalInput").ap()
    adjT = nc.dram_tensor("adjT", [B_PER_CORE, N, N], I32, kind="ExternalInput").ap()
    w = nc.dram_tensor("w", [N_HEAD, F_IN, F_OUT], F32, kind="ExternalInput").ap()
    a = nc.dram_tensor("a", [N_HEAD, F_OUT, 2], F32, kind="ExternalInput").ap()
    out = nc.dram_tensor("out", [B_PER_CORE, 128, N_HEAD, NCH, F_OUT], F32,
                         kind="ExternalOutput").ap()
    with tile.TileContext(nc) as tc:
        _gat_tile_kernel(tc, out, hT, adjT, w, a)
    nc.compile()
    return nc


_NC_CACHE = []


def _get_nc():
    if not _NC_CACHE:
        _NC_CACHE.append(_build_nc())
    return _NC_CACHE[0]


def make_in_maps(h, adj, w, a_src, a_dst, bias):
    h = np.asarray(h, dtype=np.float32)
    adj = np.asarray(adj, dtype=np.int32)
    w = np.ascontiguousarray(np.asarray(w, dtype=np.float32))
    a = np.ascontiguousarray(
        np.concatenate([np.asarray(a_src, np.float32),
                        np.asarray(a_dst, np.float32)], axis=2))
    hT = np.ascontiguousarray(h.transpose(0, 2, 1))       # [bs, f_in, n]
    adjT = np.ascontiguousarray(adj.transpose(0, 2, 1))   # [bs, n(j), n(i)]
    in_maps = []
    for c in range(N_CORES):
        sl = slice(B_PER_CORE * c, B_PER_CORE * (c + 1))
        in_maps.append({"hT": hT[sl], "adjT": adjT[sl], "w": w, "a": a})
    return in_maps


def kernel(h, adj, w, a_src, a_dst, bias):
    nc = _get_nc()
    in_maps = make_in_maps(h, adj, w, a_src, a_dst, bias)
    res = run_bass_kernel_spmd(nc, in_maps, core_ids=list(range(N_CORES)))
    out = np.concatenate([res.results[c]["out"] for c in range(N_CORES)], axis=0)
    # device layout [b, p, h, c, o] -> [b, h, c*128+p, o]
    out = out.transpose(0, 2, 3, 1, 4).reshape(BS, N_HEAD, N, F_OUT)
    out = out + np.asarray(bias, np.float32)[None, None, None, :]
    return np.ascontiguousarray(out.astype(np.float32))
